# revision 10
# baseline (speedup 1.0000x reference)
"""DeepSeekV3 block (MLA attention + top-2-of-8 MoE) on 8 trn2 NeuronCores.

Sharding: cores 0-3 -> batch 0, cores 4-7 -> batch 1. Within a batch group
of 4 cores, each core owns S/4 query tokens chosen as SL strided 128-row
blocks ordered by causal depth (blocks r+12, r+8, r+4, r for sub-rank r at
S=2048), which makes the flash-attention k-loop narrow uniformly across
cores: one SPMD program, all per-core differences live in input data
(token slices, rope tables, causal masks). k/v/latent projections are
recomputed per core (replicated within the batch group) to avoid
collectives. MoE runs dense over all 8 experts with gates folded into the
expert hidden states before the w2 contraction.

Precision: the attention path stays bf16 (the top-2 router downstream is
tie-sensitive, so x2 must match the reference closely; fp8 there flips
expert selections). The MoE expert path (h1/h3/w2 and its activations)
is post-router and runs in fp8e4m3 with DoubleRow matmuls (2x128
contraction per pass); fp8 weights are pre-scaled x32 on the host and
descaled through the free affine of evacuation copies. Flash scores use
64x128 row-tiled matmuls (two heads on separate PE array halves). The
baseline's K=1 broadcast matmuls are replaced by gpsimd
partition_broadcast, and the flash hp-loop is software-pipelined so the
scalar engine's exp stream never starves.

Layout convention: activations are kept transposed [feature, token] so
weight matrices are always the stationary matmul operand, and softmax
denominators come from a ones column appended to the value tiles.
"""

import sys

sys.path.insert(0, "/opt/trn_rl_repo")

from contextlib import ExitStack

import ml_dtypes
import numpy as np

import concourse.bass as bass
import concourse.tile as tile
from concourse import bacc
from concourse import mybir
from concourse.bass_utils import run_bass_kernel_spmd

F32 = mybir.dt.float32
BF16 = mybir.dt.bfloat16
FP8 = mybir.dt.float8e4
AF = mybir.ActivationFunctionType
ALU = mybir.AluOpType
DR = mybir.MatmulPerfMode.DoubleRow
BF = ml_dtypes.bfloat16
E4 = ml_dtypes.float8_e4m3

B, D = 2, 1024
H, HD = 16, 64
HALF = HD // 2
R = 256
E, TOPK, MH = 8, 2, 256
EPS = 1e-6
THETA = 10000.0
P = 128
NCORES = 8
WS = 32.0          # fp8 weight pre-scale (MoE experts only)
IWS = 1.0 / WS
VS = 4.0           # fp8 range scale on gated hidden states


def _build(S: int):
    NB = S // P               # seq blocks per batch (16 at S=2048)
    SL = NB // 4              # q-block slots per core
    TOK = SL * P              # own tokens per core
    WIN = min(512, S)
    NW = S // WIN
    NHP = H // 2              # 8 head pairs
    DCH = D // P              # 8
    RCH = R // P              # 2
    HD1 = HD + 1
    # flash pair column offsets in the packed e2 tile
    pairN = [(SL - (2 * jp) // 4) * P for jp in range(NB // 2)]
    pairOff = [sum(pairN[:jp]) for jp in range(NB // 2)]
    E2W = sum(pairN)          # 2560 at S=2048

    nc = bacc.Bacc(None, target_bir_lowering=False)

    xT = nc.dram_tensor("xT", [D, S], F32, kind="ExternalInput")
    xTq = nc.dram_tensor("xTq", [D, TOK], F32, kind="ExternalInput")
    cos4k = nc.dram_tensor("cos4k", [P, S], BF16, kind="ExternalInput")
    sin4kn = nc.dram_tensor("sin4kn", [P, S], BF16, kind="ExternalInput")
    cos4q = nc.dram_tensor("cos4q", [P, TOK], BF16, kind="ExternalInput")
    sin4qn = nc.dram_tensor("sin4qn", [P, TOK], BF16, kind="ExternalInput")
    maskt = nc.dram_tensor("maskt", [NB, 2, P, P], BF16, kind="ExternalInput")
    wqn = nc.dram_tensor("wqn", [D, H * HD], BF16, kind="ExternalInput")
    wdkvn = nc.dram_tensor("wdkvn", [D, R], BF16, kind="ExternalInput")
    wuk = nc.dram_tensor("wuk", [R, H * HD], BF16, kind="ExternalInput")
    wuv = nc.dram_tensor("wuv", [R, H * HD], BF16, kind="ExternalInput")
    wo = nc.dram_tensor("wo", [H * HD, D], BF16, kind="ExternalInput")
    wrn = nc.dram_tensor("wrn", [D, E], F32, kind="ExternalInput")
    bias128 = nc.dram_tensor("bias128", [P, E], F32, kind="ExternalInput")
    w13n = nc.dram_tensor("w13n", [E, D, 2 * MH], FP8, kind="ExternalInput")
    w2s = nc.dram_tensor("w2s", [E, MH, D], FP8, kind="ExternalInput")
    identf = nc.dram_tensor("identf", [P, P], F32, kind="ExternalInput")
    outT = nc.dram_tensor("outT", [D, TOK], F32, kind="ExternalOutput")

    with tile.TileContext(nc) as tc, ExitStack() as ctx:
        p_const = ctx.enter_context(tc.tile_pool(name="const", bufs=1))
        p_x2 = ctx.enter_context(tc.tile_pool(name="x2", bufs=1))

        ones_bf = p_const.tile([P, 1], BF16, tag="ones_bf", name="ones_bf")
        nc.vector.memset(ones_bf, 1.0)
        sb_mask = p_const.tile([P, NB, 2, P], BF16, tag="mask", name="mask")
        nc.sync.dma_start(sb_mask,
                          maskt[:, :, :, :].rearrange("j g k q -> k j g q"))
        sb_cos4q = p_const.tile([P, 2, TOK], BF16, tag="cos4q", name="cos4q")
        nc.sync.dma_start(sb_cos4q[:, 0, :], cos4q[:, :])
        nc.sync.dma_start(sb_cos4q[:, 1, :], cos4q[:, :])
        sb_sin4qn = p_const.tile([P, 2, TOK], BF16, tag="sin4qn",
                                 name="sin4qn")
        nc.sync.dma_start(sb_sin4qn[:, 0, :], sin4qn[:, :])
        nc.sync.dma_start(sb_sin4qn[:, 1, :], sin4qn[:, :])
        sb_bias = p_const.tile([P, E], F32, tag="bias", name="bias")
        nc.sync.dma_start(sb_bias, bias128[:, :])
        sb_ident = p_const.tile([P, P], F32, tag="ident", name="ident")
        nc.sync.dma_start(sb_ident, identf[:, :])
        sb_wrn = p_const.tile([P, DCH, E], F32, tag="wrn", name="wrn")
        nc.sync.dma_start(sb_wrn, wrn[:, :].rearrange("(c p) e -> p c e", p=P))
        eps1 = p_const.tile([1, 1], F32, tag="eps1", name="eps1")
        nc.vector.memset(eps1, EPS)

        sb_xq = []
        for dch in range(DCH):
            t = p_x2.tile([P, TOK], F32, tag=f"xq{dch}", name=f"xq{dch}")
            nc.sync.dma_start(t, xTq[dch * P:(dch + 1) * P, :])
            sb_xq.append(t)

        def rmsnorm_cols(pool, ppool, src_tiles, ncols, nametag, dtype):
            """src_tiles: DCH sbuf [P, ncols] f32 -> one [P, DCH, ncols]
            tile of `dtype`, rms-normalized across the full d axis, plus
            the f32 [P, ncols] 1/rms broadcast tile."""
            ss = ppool.tile([1, ncols], F32, tag="ss", name="ss")
            sq = []
            for dch in range(DCH):
                t = pool.tile([P, ncols], BF16, tag=f"sq{dch % 2}",
                              name=f"sq{dch % 2}")
                nc.gpsimd.tensor_tensor(t, src_tiles[dch], src_tiles[dch],
                                        ALU.mult)
                sq.append(t)
            for dch in range(DCH):
                nc.tensor.matmul(ss, ones_bf, sq[dch],
                                 start=(dch == 0), stop=(dch == DCH - 1))
            sd = pool.tile([1, ncols], F32, tag="sd", name="sd")
            nc.scalar.activation(sd, ss, AF.Sqrt, bias=eps1, scale=1.0 / D)
            sdw = pool.tile([P, ncols // P], F32, tag="sdw", name="sdw")
            nc.sync.dma_start(sdw, sd)
            rcw = pool.tile([P, ncols // P], F32, tag="rcw", name="rcw")
            nc.vector.reciprocal(rcw, sdw)
            rsv = pool.tile([1, ncols], F32, tag="rsv", name="rsv")
            nc.sync.dma_start(rsv, rcw)
            rsb = pool.tile([P, ncols], F32, tag=f"rsb_{nametag}",
                            name=f"rsb_{nametag}")
            nc.gpsimd.partition_broadcast(rsb, rsv)
            h = pool.tile([P, DCH, ncols], dtype, tag=f"h_{nametag}",
                          name=f"h_{nametag}")
            for dch in range(DCH):
                eng = nc.vector if dch % 2 == 0 else nc.gpsimd
                eng.tensor_tensor(h[:, dch, :], src_tiles[dch], rsb,
                                  ALU.mult)
            return h, rsb

        def rope6(pool, pre_ps, cos_t, sin_t, out_ap, nametag, evac):
            """rope on 2D psum [P, cols] (head pairs stacked) -> bf16
            out_ap. Engines are partition-lane-locked, so the half-swap
            goes through SBUF->SBUF DMA."""
            shp = [int(s) for s in pre_ps.shape]
            kbf = pool.tile(shp, BF16, tag=f"rkb_{nametag}",
                            name=f"rkb_{nametag}")
            if evac == "scalar":
                nc.scalar.copy(kbf, pre_ps)
            else:
                nc.vector.tensor_scalar(kbf, pre_ps, 1.0, None, ALU.mult)
            ksw = pool.tile(shp, BF16, tag=f"rsw_{nametag}",
                            name=f"rsw_{nametag}")
            for g in range(4):
                a = g * HALF
                pa = (g + 1) * HALF if g % 2 == 0 else (g - 1) * HALF
                nc.sync.dma_start(ksw[a:a + HALF], kbf[pa:pa + HALF])
            tmp = pool.tile(shp, BF16, tag=f"rtm_{nametag}",
                            name=f"rtm_{nametag}")
            nc.gpsimd.tensor_tensor(tmp, ksw, sin_t, ALU.mult)
            nc.vector.tensor_tensor(out_ap, kbf, cos_t, ALU.mult)
            nc.gpsimd.tensor_tensor(out_ap, out_ap, tmp, ALU.add)

        attnT = p_x2.tile([P, NHP, TOK], BF16, tag="attnT", name="attnT")
        qTa = p_x2.tile([P, NHP, TOK], BF16, tag="qTa", name="qTa")

        with ExitStack() as kvctx:
            p_kv = kvctx.enter_context(tc.tile_pool(name="kv", bufs=1))
            vextT = p_kv.tile([P, NB, H, HD1], BF16, tag="vextT",
                              name="vextT")
            cT = p_kv.tile([P, RCH, S], BF16, tag="cT", name="cT")

            # ---- phase 1+2: rmsnorm of full batch -> latent cT ----
            with ExitStack() as s12:
                p_xw = s12.enter_context(tc.tile_pool(name="xw", bufs=2))
                p_n1 = s12.enter_context(tc.tile_pool(name="n1", bufs=2))
                p_wd = s12.enter_context(tc.tile_pool(name="wd", bufs=1))
                pp_12 = s12.enter_context(
                    tc.tile_pool(name="p12", bufs=2, space="PSUM"))
                sb_wdkv = p_wd.tile([P, DCH, R], BF16, tag="wdkv",
                                    name="wdkv")
                nc.sync.dma_start(
                    sb_wdkv, wdkvn[:, :].rearrange("(c p) r -> p c r", p=P))
                for w in range(NW):
                    c0 = w * WIN
                    xw = []
                    for dch in range(DCH):
                        t = p_xw.tile([P, WIN], F32, tag=f"xw{dch}",
                                      name=f"xw{dch}")
                        nc.sync.dma_start(
                            t, xT[dch * P:(dch + 1) * P, c0:c0 + WIN])
                        xw.append(t)
                    h1w, _ = rmsnorm_cols(p_n1, pp_12, xw, WIN, "n1", BF16)
                    for rch in range(RCH):
                        cps = pp_12.tile([P, WIN], F32, tag="mm", name="mm")
                        for dch in range(DCH):
                            nc.tensor.matmul(
                                cps,
                                sb_wdkv[:, dch, rch * P:(rch + 1) * P],
                                h1w[:, dch, :],
                                start=(dch == 0), stop=(dch == DCH - 1))
                        if rch == 0:
                            nc.scalar.copy(cT[:, rch, c0:c0 + WIN], cps)
                        else:
                            nc.vector.tensor_scalar(cT[:, rch, c0:c0 + WIN],
                                                    cps, 1.0, None, ALU.mult)

            # ---- phase 4: v -> vextT (ones in column HD of each head) ----
            with ExitStack() as s4:
                p_wv = s4.enter_context(tc.tile_pool(name="wv", bufs=1))
                pp_4 = s4.enter_context(
                    tc.tile_pool(name="p4", bufs=2, space="PSUM"))
                sb_wuv = p_wv.tile([P, RCH, H * HD], BF16, tag="wuv",
                                   name="wuv")
                nc.sync.dma_start(
                    sb_wuv, wuv[:, :].rearrange("(c p) n -> p c n", p=P))
                for tb2 in range(NB // 2):
                    for nh in range(2):
                        vps = pp_4.tile([P, 2, 512], F32, tag="mm", name="mm")
                        for half in range(2):
                            tb = 2 * tb2 + half
                            for rch in range(RCH):
                                nc.tensor.matmul(
                                    vps[:, half, :],
                                    cT[:, rch, tb * P:(tb + 1) * P],
                                    sb_wuv[:, rch, nh * 512:(nh + 1) * 512],
                                    start=(rch == 0), stop=(rch == RCH - 1))
                        dst = vextT[:, 2 * tb2:2 * tb2 + 2,
                                    nh * 8:(nh + 1) * 8, 0:HD]
                        src = vps[:, :, :].rearrange(
                            "p t (h s) -> p t h s", s=HD)
                        if nh == 0:
                            nc.scalar.copy(dst, src)
                        else:
                            nc.vector.tensor_scalar(dst, src, 1.0, None,
                                                    ALU.mult)
                nc.vector.memset(vextT[:, :, :, HD:HD1], 1.0)

            # ---- phase 5: qT + rope (own tokens) ----
            with ExitStack() as s5:
                p_q = s5.enter_context(tc.tile_pool(name="q", bufs=2))
                p_wq = s5.enter_context(tc.tile_pool(name="wqp", bufs=1))
                pp_5 = s5.enter_context(
                    tc.tile_pool(name="p5", bufs=2, space="PSUM"))
                sb_wq = p_wq.tile([P, DCH, H * HD], BF16, tag="wq", name="wq")
                nc.sync.dma_start(
                    sb_wq, wqn[:, :].rearrange("(c p) n -> p c n", p=P))
                h1q, _ = rmsnorm_cols(p_q, pp_5, sb_xq, TOK, "nq", BF16)
                for hp2 in range(NHP // 2):
                    qps = pp_5.tile([P, 2, TOK], F32, tag="mm2", name="mm2")
                    for half in range(2):
                        hc = (2 * hp2 + half) * 2 * HD
                        for dch in range(DCH):
                            nc.tensor.matmul(
                                qps[:, half, :],
                                sb_wq[:, dch, hc:hc + P],
                                h1q[:, dch, :],
                                start=(dch == 0), stop=(dch == DCH - 1))
                    rope6(p_q,
                          qps[:, :, :].rearrange("p a t -> p (a t)"),
                          sb_cos4q[:, :, :].rearrange("p a t -> p (a t)"),
                          sb_sin4qn[:, :, :].rearrange("p a t -> p (a t)"),
                          qTa[:, 2 * hp2:2 * hp2 + 2, :].rearrange(
                              "p a t -> p (a t)"),
                          "q", evac="scalar")

            # ---- phase 3: all kT + rope, precomputed ----
            ktA = p_kv.tile([P, NHP, S], BF16, tag="ktA", name="ktA")
            with ExitStack() as s3:
                p_kr = s3.enter_context(tc.tile_pool(name="kr", bufs=2))
                p_wk = s3.enter_context(tc.tile_pool(name="wk", bufs=1))
                pp_3 = s3.enter_context(
                    tc.tile_pool(name="p3", bufs=2, space="PSUM"))
                sb_wuk = p_wk.tile([P, RCH, H * HD], BF16, tag="wuk",
                                   name="wuk")
                nc.sync.dma_start(
                    sb_wuk, wuk[:, :].rearrange("(c p) n -> p c n", p=P))
                sb_cos4k = p_wk.tile([P, S], BF16, tag="cos4k", name="cos4k")
                nc.sync.dma_start(sb_cos4k, cos4k[:, :])
                sb_sin4kn = p_wk.tile([P, S], BF16, tag="sin4kn",
                                      name="sin4kn")
                nc.sync.dma_start(sb_sin4kn, sin4kn[:, :])
                for hp in range(NHP):
                    hc = hp * 2 * HD
                    for w2i in range(NW // 2):
                        c0 = w2i * 2 * WIN
                        kps = pp_3.tile([P, 2, WIN], F32, tag="kps",
                                        name="kps")
                        for half in range(2):
                            cw = c0 + half * WIN
                            for rch in range(RCH):
                                nc.tensor.matmul(
                                    kps[:, half, :],
                                    sb_wuk[:, rch, hc:hc + P],
                                    cT[:, rch, cw:cw + WIN],
                                    start=(rch == 0), stop=(rch == RCH - 1))
                        rope6(p_kr,
                              kps[:, :, :].rearrange("p a t -> p (a t)"),
                              sb_cos4k[:, c0:c0 + 2 * WIN],
                              sb_sin4kn[:, c0:c0 + 2 * WIN],
                              ktA[:, hp, c0:c0 + 2 * WIN],
                              "k", evac="vector" if hp % 2 else "scalar")

            # ---- phase 6: flash, software-pipelined over head pairs:
            # scores(hp) stream on scalar(exp) while values(hp-1) run on
            # the tensor engine ----
            with ExitStack() as s6:
                p_fl = s6.enter_context(tc.tile_pool(name="fl", bufs=2))
                p_e2 = s6.enter_context(tc.tile_pool(name="e2p", bufs=2))
                pp_s = s6.enter_context(
                    tc.tile_pool(name="psc", bufs=3, space="PSUM"))
                pp_o = s6.enter_context(
                    tc.tile_pool(name="po", bufs=1, space="PSUM"))

                def scores_phase(hp):
                    e2 = p_e2.tile([P, 4, E2W], BF16, tag="e2", name="e2")
                    for jp in range(NB // 2):
                        N = pairN[jp]
                        cc = pairOff[jp]
                        for dj in range(2):
                            j = 2 * jp + dj
                            jc = slice(j * P, (j + 1) * P)
                            s2 = pp_s.tile([P, 2, 512], F32, tag="s2",
                                           name="s2")
                            nc.tensor.matmul(
                                s2[:, 0, 0:N], ktA[0:HD, hp, jc],
                                qTa[0:HD, hp, 0:N],
                                start=True, stop=True, tile_position=(0, 0))
                            nc.tensor.matmul(
                                s2[:, 1, 0:N], ktA[HD:P, hp, jc],
                                qTa[HD:P, hp, 0:N],
                                start=True, stop=True, tile_position=(64, 0))
                            nc.scalar.activation(
                                e2[:, 2 * dj:2 * dj + 2, cc:cc + N],
                                s2[:, :, 0:N], AF.Exp, scale=0.125)
                        nc.vector.tensor_tensor(
                            e2[:, :, cc + N - P:cc + N].rearrange(
                                "p (a b) q -> p a b q", b=2),
                            e2[:, :, cc + N - P:cc + N].rearrange(
                                "p (a b) q -> p a b q", b=2),
                            sb_mask[:, 2 * jp:2 * jp + 2, :, :],
                            ALU.mult)
                    return e2

                def values_phase(hp, e2):
                    O2 = pp_o.tile([P, 2, 512], F32, tag="O2", name="O2")
                    for g in range(2):
                        for jp in range(NB // 2):
                            N = pairN[jp]
                            cc = pairOff[jp]
                            for dj in range(2):
                                j = 2 * jp + dj
                                ve = vextT[:, j, hp * 2 + g, :]
                                nc.tensor.matmul(
                                    O2[0:HD1, g, 0:N], ve,
                                    e2[:, 2 * dj + g, cc:cc + N],
                                    start=(jp == 0 and dj == 0),
                                    stop=(jp == NB // 2 - 1 and dj == 1),
                                    skip_group_check=True)
                    # normalize: attnT = O[0:64] * (1/l); l sits in row 64
                    sums = p_fl.tile([1, 2, TOK], F32, tag="sums",
                                     name="sums")
                    nc.scalar.copy(sums, O2[HD:HD1, :, 0:TOK])
                    sw = p_fl.tile([P, 2 * TOK // P], F32, tag="sw",
                                   name="sw")
                    nc.sync.dma_start(sw, sums)
                    rw = p_fl.tile([P, 2 * TOK // P], F32, tag="rw",
                                   name="rw")
                    nc.vector.reciprocal(rw, sw)
                    linv = p_fl.tile([1, 2, TOK], F32, tag="linv",
                                     name="linv")
                    nc.sync.dma_start(linv, rw)
                    lb = p_fl.tile([P, 2, TOK], F32, tag="lb", name="lb")
                    nc.gpsimd.partition_broadcast(lb[0:HD, :, :],
                                                  linv, channels=HD)
                    nc.vector.tensor_tensor(attnT[0:HD, hp, :],
                                            O2[0:HD, 0, 0:TOK],
                                            lb[0:HD, 0, :], ALU.mult)
                    a2 = p_fl.tile([HD, TOK], BF16, tag="a2", name="a2")
                    nc.vector.tensor_tensor(a2, O2[0:HD, 1, 0:TOK],
                                            lb[0:HD, 1, :], ALU.mult)
                    # head 2 belongs on partitions 64:128 -> move via DMA
                    nc.sync.dma_start(attnT[HD:P, hp, :], a2)

                e2_prev = scores_phase(0)
                for hp in range(1, NHP):
                    e2_cur = scores_phase(hp)
                    values_phase(hp - 1, e2_prev)
                    e2_prev = e2_cur
                values_phase(NHP - 1, e2_prev)

        # ---- phase 7: wo + residual -> x2T ----
        x2T = [p_x2.tile([P, TOK], F32, tag=f"x2T{i}", name=f"x2T{i}")
               for i in range(DCH)]
        with ExitStack() as s7:
            p_wo = s7.enter_context(tc.tile_pool(name="wop", bufs=1))
            pp_wo = s7.enter_context(
                tc.tile_pool(name="pwo", bufs=2, space="PSUM"))
            sb_wo = p_wo.tile([P, DCH, D], BF16, tag="wo", name="wo")
            nc.sync.dma_start(
                sb_wo, wo[:, :].rearrange("(c p) n -> p c n", p=P))
            for dch in range(DCH):
                yps = pp_wo.tile([P, TOK], F32, tag="yps", name="yps")
                for hch in range(DCH):
                    nc.tensor.matmul(
                        yps, sb_wo[:, hch, dch * P:(dch + 1) * P],
                        attnT[:, hch, :],
                        start=(hch == 0), stop=(hch == DCH - 1))
                nc.vector.tensor_tensor(x2T[dch], yps, sb_xq[dch], ALU.add)

        # ================= MoE =================
        with ExitStack() as mctx:
            p_moe = mctx.enter_context(tc.tile_pool(name="moe", bufs=1))
            p_sm = mctx.enter_context(tc.tile_pool(name="sm", bufs=2))

            gatesT = p_moe.tile([E, TOK], BF16, tag="gatesT", name="gatesT")
            with ExitStack() as rctx:
                pp_r = rctx.enter_context(
                    tc.tile_pool(name="pr", bufs=2, space="PSUM"))
                x2n8, rsb2 = rmsnorm_cols(p_moe, pp_r, x2T, TOK, "n2", FP8)
                # fp32 normalized copy for the router (top-2 selection is
                # tie-sensitive; low-precision scores flip expert choices)
                x2nf = []
                for dch in range(DCH):
                    t = p_moe.tile([P, TOK], F32, tag=f"x2nf{dch}",
                                   name=f"x2nf{dch}")
                    eng = nc.vector if dch % 2 == 0 else nc.gpsimd
                    eng.tensor_tensor(t, x2T[dch], rsb2, ALU.mult)
                    x2nf.append(t)
                scp = pp_r.tile([E, TOK], F32, tag="scp", name="scp")
                for dch in range(DCH):
                    nc.tensor.matmul(scp, sb_wrn[:, dch, :], x2nf[dch],
                                     start=(dch == 0),
                                     stop=(dch == DCH - 1))
                sg8 = p_moe.tile([E, TOK], F32, tag="sg8", name="sg8")
                nc.scalar.activation(sg8, scp, AF.Sigmoid)
                for tb in range(SL):
                    tcs = slice(tb * P, (tb + 1) * P)
                    sgt = pp_r.tile([P, E], F32, tag="sgt", name="sgt")
                    nc.tensor.transpose(sgt, sg8[:, tcs],
                                        sb_ident[0:E, 0:E])
                    tt = p_sm.tile([P, E], F32, tag="tt", name="tt")
                    nc.vector.tensor_tensor(tt, sgt, sb_bias, ALU.add)
                    m1 = p_sm.tile([P, 1], F32, tag="m1", name="m1")
                    nc.vector.tensor_reduce(m1, tt, mybir.AxisListType.X,
                                            ALU.max)
                    e1 = p_sm.tile([P, E], F32, tag="e1", name="e1")
                    nc.vector.tensor_scalar(e1, tt, m1, None, ALU.is_ge)
                    t2 = p_sm.tile([P, E], F32, tag="t2", name="t2")
                    nc.vector.scalar_tensor_tensor(t2, e1, -1e9, tt,
                                                   ALU.mult, ALU.add)
                    m2 = p_sm.tile([P, 1], F32, tag="m2", name="m2")
                    nc.vector.tensor_reduce(m2, t2, mybir.AxisListType.X,
                                            ALU.max)
                    e2g = p_sm.tile([P, E], F32, tag="e2g", name="e2g")
                    nc.vector.tensor_scalar(e2g, t2, m2, None, ALU.is_ge)
                    sel = p_sm.tile([P, E], F32, tag="sel", name="sel")
                    nc.vector.tensor_tensor(sel, e1, e2g, ALU.add)
                    gg = p_sm.tile([P, E], F32, tag="gg", name="gg")
                    nc.vector.tensor_tensor(gg, sgt, sel, ALU.mult)
                    dsum = p_sm.tile([P, 1], F32, tag="dsum", name="dsum")
                    nc.vector.tensor_reduce(dsum, gg, mybir.AxisListType.X,
                                            ALU.add)
                    nc.vector.tensor_scalar(dsum, dsum, 1e-9, None, ALU.add)
                    rcp = p_sm.tile([P, 1], F32, tag="rcp", name="rcp")
                    nc.vector.reciprocal(rcp, dsum)
                    nc.vector.tensor_scalar(gg, gg, rcp, None, ALU.mult)
                    gtp = pp_r.tile([E, P], F32, tag="gtp", name="gtp")
                    nc.tensor.transpose(gtp, gg, sb_ident)
                    nc.scalar.copy(gatesT[:, tcs], gtp)

            # gated expert hidden states, fp8 (VS x h1s*h3*gate)
            h2g = p_moe.tile([P, E, 2, TOK], FP8, tag="h2g", name="h2g")
            with ExitStack() as ectx:
                p_mw = ectx.enter_context(tc.tile_pool(name="mw", bufs=3))
                pp_h = ectx.enter_context(
                    tc.tile_pool(name="phps", bufs=2, space="PSUM"))
                for e in range(E):
                    w13t = p_mw.tile([P, DCH, 2 * MH], FP8, tag="w13t",
                                     name="w13t")
                    nc.sync.dma_start(
                        w13t,
                        w13n[e, :, :].rearrange("(c p) n -> p c n", p=P))
                    ge = p_sm.tile([1, TOK], BF16, tag="ge", name="ge")
                    nc.sync.dma_start(ge, gatesT[e:e + 1, :])
                    gb = p_sm.tile([P, TOK], BF16, tag="gb", name="gb")
                    nc.gpsimd.partition_broadcast(gb, ge)
                    hpre = []
                    for m in range(4):
                        hps = pp_h.tile([P, TOK], F32, tag=f"hps{m}",
                                        name=f"hps{m}")
                        for dp in range(DCH // 2):
                            nc.tensor.matmul(
                                hps,
                                w13t[:, 2 * dp:2 * dp + 2,
                                     m * P:(m + 1) * P],
                                x2n8[:, 2 * dp:2 * dp + 2, :],
                                start=(dp == 0), stop=(dp == DCH // 2 - 1),
                                perf_mode=DR)
                        hpre.append(hps)
                    for m in range(2):
                        sl = p_sm.tile([P, TOK], BF16, tag="sl", name="sl")
                        nc.scalar.activation(sl, hpre[m], AF.Silu,
                                             scale=IWS)
                        tg = p_sm.tile([P, TOK], BF16, tag="tg", name="tg")
                        nc.vector.scalar_tensor_tensor(
                            tg, hpre[m + 2], VS * IWS, sl,
                            ALU.mult, ALU.mult)
                        nc.gpsimd.tensor_tensor(h2g[:, e, m, :], tg, gb,
                                                ALU.mult)

            with ExitStack() as w2ctx:
                p_w2 = w2ctx.enter_context(tc.tile_pool(name="w2p", bufs=1))
                pp_yf = w2ctx.enter_context(
                    tc.tile_pool(name="pyf", bufs=2, space="PSUM"))
                w2all = []
                for e in range(E):
                    t = p_w2.tile([P, 2, D], FP8, tag=f"w2_{e}",
                                  name=f"w2_{e}")
                    nc.sync.dma_start(
                        t, w2s[e, :, :].rearrange("(c p) n -> p c n", p=P))
                    w2all.append(t)
                for dch in range(DCH):
                    yf = pp_yf.tile([P, TOK], F32, tag="yf", name="yf")
                    for e in range(E):
                        nc.tensor.matmul(
                            yf, w2all[e][:, :, dch * P:(dch + 1) * P],
                            h2g[:, e, :, :],
                            start=(e == 0), stop=(e == E - 1),
                            perf_mode=DR)
                    ot = p_sm.tile([P, TOK], F32, tag="ot", name="ot")
                    nc.vector.scalar_tensor_tensor(
                        ot, yf, 1.0 / (WS * VS), x2T[dch],
                        ALU.mult, ALU.add)
                    nc.sync.dma_start(outT[dch * P:(dch + 1) * P, :], ot)

    nc.compile()
    return nc


_NC_CACHE = {}


def _get_nc(S):
    if S not in _NC_CACHE:
        _NC_CACHE[S] = _build(S)
    return _NC_CACHE[S]


def host_prep(x, position_ids, norm1_w, wq, wdkv, wuk, wuv, wo,
              norm2_w, wr, router_bias, w1, w3, w2):
    x = np.asarray(x, np.float32)
    _, S, _ = x.shape
    NB = S // P
    SL = NB // 4

    pos = np.asarray(position_ids, np.int32)
    norm1_w = np.asarray(norm1_w, np.float32)
    norm2_w = np.asarray(norm2_w, np.float32)
    wq_n = (np.asarray(wq, np.float32) * norm1_w[:, None]).astype(BF)
    wdkv_n = (np.asarray(wdkv, np.float32) * norm1_w[:, None]).astype(BF)
    wuk_b = np.asarray(wuk, np.float32).astype(BF)
    wuv_b = np.asarray(wuv, np.float32).astype(BF)
    wo_b = np.asarray(wo, np.float32).astype(BF)
    wr_n = np.ascontiguousarray(np.asarray(wr, np.float32) * norm2_w[:, None])
    w13 = np.concatenate([np.asarray(w1, np.float32),
                          np.asarray(w3, np.float32)], axis=2)
    w13_n = np.ascontiguousarray(
        w13 * norm2_w[None, :, None] * WS).astype(E4)
    w2_b = np.ascontiguousarray(np.asarray(w2, np.float32) * WS).astype(E4)
    bias_b = np.ascontiguousarray(np.broadcast_to(
        np.asarray(router_bias, np.float32)[None, :], (P, E)))
    ident = np.eye(P, dtype=np.float32)

    inv = 1.0 / (THETA ** (np.arange(HALF, dtype=np.float64) / HALF))

    in_maps = []
    slot_blocks_all = []
    for c in range(NCORES):
        b, r = divmod(c, 4)
        slot_blocks = [r + 4 * (SL - 1 - m) for m in range(SL)]
        slot_blocks_all.append(slot_blocks)
        own = np.concatenate(
            [np.arange(g * P, (g + 1) * P) for g in slot_blocks])

        ang = pos[b].astype(np.float64)[:, None] * inv[None, :]
        cosT = np.cos(ang).T.astype(np.float32)
        sinT = np.sin(ang).T.astype(np.float32)
        cos4k_h = np.tile(cosT, (4, 1)).astype(BF)
        sin4kn_h = np.concatenate([-sinT, sinT, -sinT, sinT], 0).astype(BF)
        cos4q_h = np.ascontiguousarray(cos4k_h[:, own])
        sin4qn_h = np.ascontiguousarray(sin4kn_h[:, own])

        xT_h = np.ascontiguousarray(x[b].T)
        xTq_h = np.ascontiguousarray(x[b].T[:, own])

        maskt_h = np.zeros((NB, P, P), np.float32)
        for j in range(NB):
            jm = j % 4
            if jm < r:
                maskt_h[j] = 1.0
            elif jm == r:
                maskt_h[j] = np.triu(np.ones((P, P), np.float32))
        maskt_h = np.repeat(maskt_h[:, None, :, :], 2, axis=1).astype(BF)

        in_maps.append({
            "xT": xT_h, "xTq": xTq_h,
            "cos4k": cos4k_h, "sin4kn": sin4kn_h,
            "cos4q": cos4q_h, "sin4qn": sin4qn_h,
            "maskt": maskt_h,
            "wqn": wq_n, "wdkvn": wdkv_n, "wuk": wuk_b, "wuv": wuv_b,
            "wo": wo_b, "wrn": wr_n, "bias128": bias_b,
            "w13n": w13_n, "w2s": w2_b, "identf": ident,
        })
    return in_maps, slot_blocks_all


def run(inputs, trace=False):
    x = np.asarray(inputs["x"], np.float32)
    Bx, S, Dx = x.shape
    nc = _get_nc(S)
    in_maps, slot_blocks_all = host_prep(**inputs)
    res = run_bass_kernel_spmd(nc, in_maps, core_ids=list(range(NCORES)),
                               trace=trace)
    out = np.zeros((Bx, S, Dx), np.float32)
    for c in range(NCORES):
        b = c // 4
        oT = np.asarray(res.results[c]["outT"])
        for m, g in enumerate(slot_blocks_all[c]):
            out[b, g * P:(g + 1) * P, :] = oT[:, m * P:(m + 1) * P].T
    return out, res


def kernel(**inputs):
    out, _ = run(inputs)
    return out


# revision 15
# speedup vs baseline: 1.0313x; 1.0313x over previous
"""DeepSeekV3 block (MLA attention + top-2-of-8 MoE) on 8 trn2 NeuronCores.

Sharding: cores 0-3 -> batch 0, cores 4-7 -> batch 1. Within a batch group
of 4 cores, each core owns S/4 query tokens chosen as SL strided 128-row
blocks ordered by causal depth (blocks r+12, r+8, r+4, r for sub-rank r at
S=2048), which makes the flash-attention k-loop narrow uniformly across
cores: one SPMD program, all per-core differences live in input data
(token slices, rope tables, causal masks). k/v/latent projections are
recomputed per core (replicated within the batch group) to avoid
collectives. MoE runs dense over all 8 experts with gates folded into the
expert hidden states before the w2 contraction.

Precision: the attention path stays bf16 (the top-2 router downstream is
tie-sensitive, so x2 must match the reference closely; fp8 there flips
expert selections). The MoE expert path (h1/h3/w2 and its activations)
is post-router and runs in fp8e4m3 with DoubleRow matmuls (2x128
contraction per pass); fp8 weights are pre-scaled x32 on the host and
descaled through the free affine of evacuation copies.

Scheduling: rmsnorm 1/rms factors are FOLDED into psum evacuations
(projections consume raw bf16 activations), so the sqrt/recip/broadcast
chain runs parallel to the matmuls instead of gating them. Flash scores
use 64x128 row-tiled matmuls (two heads on separate PE array halves),
and the flash hp-loop is software-pipelined: scores(hp) keep the scalar
engine's exp stream fed while values(hp-1) and kproj+rope(hp+1) run on
the tensor/vector engines. The baseline's K=1 broadcast matmuls are
replaced by gpsimd partition_broadcast.

Layout convention: activations are kept transposed [feature, token] so
weight matrices are always the stationary matmul operand, and softmax
denominators come from a ones column appended to the value tiles.
"""

import sys

sys.path.insert(0, "/opt/trn_rl_repo")

from contextlib import ExitStack

import ml_dtypes
import numpy as np

import concourse.bass as bass
import concourse.tile as tile
from concourse import bacc
from concourse import mybir
from concourse.bass_utils import run_bass_kernel_spmd

F32 = mybir.dt.float32
BF16 = mybir.dt.bfloat16
FP8 = mybir.dt.float8e4
AF = mybir.ActivationFunctionType
ALU = mybir.AluOpType
DR = mybir.MatmulPerfMode.DoubleRow
BF = ml_dtypes.bfloat16
E4 = ml_dtypes.float8_e4m3

B, D = 2, 1024
H, HD = 16, 64
HALF = HD // 2
R = 256
E, TOPK, MH = 8, 2, 256
EPS = 1e-6
THETA = 10000.0
P = 128
NCORES = 8
WS = 32.0          # fp8 weight pre-scale (MoE experts only)
IWS = 1.0 / WS
VS = 4.0           # fp8 range scale on gated hidden states


def _build(S: int):
    NB = S // P               # seq blocks per batch (16 at S=2048)
    SL = NB // 4              # q-block slots per core
    TOK = SL * P              # own tokens per core
    WIN = min(512, S)
    NW = S // WIN
    NHP = H // 2              # 8 head pairs
    DCH = D // P              # 8
    RCH = R // P              # 2
    HD1 = HD + 1
    # flash pair column offsets in the packed e2 tile
    pairN = [(SL - (2 * jp) // 4) * P for jp in range(NB // 2)]
    pairOff = [sum(pairN[:jp]) for jp in range(NB // 2)]
    E2W = sum(pairN)          # 2560 at S=2048

    nc = bacc.Bacc(None, target_bir_lowering=False)

    xTbf = nc.dram_tensor("xTbf", [D, S], BF16, kind="ExternalInput")
    xTqbf = nc.dram_tensor("xTqbf", [D, TOK], BF16, kind="ExternalInput")
    xTq = nc.dram_tensor("xTq", [D, TOK], F32, kind="ExternalInput")
    cos4k = nc.dram_tensor("cos4k", [P, S], BF16, kind="ExternalInput")
    sin4kn = nc.dram_tensor("sin4kn", [P, S], BF16, kind="ExternalInput")
    cos4q = nc.dram_tensor("cos4q", [P, TOK], BF16, kind="ExternalInput")
    sin4qn = nc.dram_tensor("sin4qn", [P, TOK], BF16, kind="ExternalInput")
    maskt = nc.dram_tensor("maskt", [NB, 2, P, P], BF16, kind="ExternalInput")
    wqn = nc.dram_tensor("wqn", [D, H * HD], BF16, kind="ExternalInput")
    wdkvn = nc.dram_tensor("wdkvn", [D, R], BF16, kind="ExternalInput")
    wuk = nc.dram_tensor("wuk", [R, H * HD], BF16, kind="ExternalInput")
    wuv = nc.dram_tensor("wuv", [R, H * HD], BF16, kind="ExternalInput")
    wo = nc.dram_tensor("wo", [H * HD, D], BF16, kind="ExternalInput")
    wrn = nc.dram_tensor("wrn", [D, E], F32, kind="ExternalInput")
    bias128 = nc.dram_tensor("bias128", [P, E], F32, kind="ExternalInput")
    w13n = nc.dram_tensor("w13n", [E, D, 2 * MH], FP8, kind="ExternalInput")
    w2s = nc.dram_tensor("w2s", [E, MH, D], FP8, kind="ExternalInput")
    identf = nc.dram_tensor("identf", [P, P], F32, kind="ExternalInput")
    outT = nc.dram_tensor("outT", [D, TOK], F32, kind="ExternalOutput")

    with tile.TileContext(nc) as tc, ExitStack() as ctx:
        p_const = ctx.enter_context(tc.tile_pool(name="const", bufs=1))
        p_x2 = ctx.enter_context(tc.tile_pool(name="x2", bufs=1))

        ones_bf = p_const.tile([P, 1], BF16, tag="ones_bf", name="ones_bf")
        nc.vector.memset(ones_bf, 1.0)
        sb_mask = p_const.tile([P, NB, 2, P], BF16, tag="mask", name="mask")
        nc.sync.dma_start(sb_mask,
                          maskt[:, :, :, :].rearrange("j g k q -> k j g q"))
        sb_cos4q = p_const.tile([P, 2, TOK], BF16, tag="cos4q", name="cos4q")
        nc.sync.dma_start(sb_cos4q[:, 0, :], cos4q[:, :])
        nc.sync.dma_start(sb_cos4q[:, 1, :], cos4q[:, :])
        sb_sin4qn = p_const.tile([P, 2, TOK], BF16, tag="sin4qn",
                                 name="sin4qn")
        nc.sync.dma_start(sb_sin4qn[:, 0, :], sin4qn[:, :])
        nc.sync.dma_start(sb_sin4qn[:, 1, :], sin4qn[:, :])
        sb_bias = p_const.tile([P, E], F32, tag="bias", name="bias")
        nc.sync.dma_start(sb_bias, bias128[:, :])
        sb_ident = p_const.tile([P, P], F32, tag="ident", name="ident")
        nc.sync.dma_start(sb_ident, identf[:, :])
        sb_wrn = p_const.tile([P, DCH, E], F32, tag="wrn", name="wrn")
        nc.sync.dma_start(sb_wrn, wrn[:, :].rearrange("(c p) e -> p c e", p=P))
        eps1 = p_const.tile([1, 1], F32, tag="eps1", name="eps1")
        nc.vector.memset(eps1, EPS)

        sb_xq = []
        for dch in range(DCH):
            t = p_x2.tile([P, TOK], F32, tag=f"xq{dch}", name=f"xq{dch}")
            nc.sync.dma_start(t, xTq[dch * P:(dch + 1) * P, :])
            sb_xq.append(t)
        sb_xqbf = p_x2.tile([P, DCH, TOK], BF16, tag="xqbf", name="xqbf")
        nc.sync.dma_start(sb_xqbf,
                          xTqbf[:, :].rearrange("(c p) t -> p c t", p=P))

        def rms_factors(pool, ppool, src_aps, ncols, nametag, sq_split=5):
            """src_aps: DCH sbuf APs [P, ncols] (bf16 or f32) -> f32
            [P, ncols] 1/rms broadcast tile + [1, ncols] row."""
            ss = ppool.tile([1, ncols], F32, tag="ss", name="ss")
            sq = []
            for dch in range(DCH):
                t = pool.tile([P, ncols], BF16, tag=f"sq{dch % 2}",
                              name=f"sq{dch % 2}")
                if dch < sq_split:
                    nc.gpsimd.tensor_tensor(t, src_aps[dch],
                                            src_aps[dch], ALU.mult)
                else:
                    nc.scalar.activation(t, src_aps[dch], AF.Square)
                sq.append(t)
            for dch in range(DCH):
                nc.tensor.matmul(ss, ones_bf, sq[dch],
                                 start=(dch == 0), stop=(dch == DCH - 1))
            sd = pool.tile([1, ncols], F32, tag="sd", name="sd")
            nc.scalar.activation(sd, ss, AF.Sqrt, bias=eps1, scale=1.0 / D)
            sdw = pool.tile([P, ncols // P], F32, tag="sdw", name="sdw")
            nc.sync.dma_start(sdw, sd)
            rcw = pool.tile([P, ncols // P], F32, tag="rcw", name="rcw")
            nc.vector.reciprocal(rcw, sdw)
            rsv = pool.tile([1, ncols], F32, tag="rsv", name="rsv")
            nc.sync.dma_start(rsv, rcw)
            rsb = pool.tile([P, ncols], F32, tag=f"rsb_{nametag}",
                            name=f"rsb_{nametag}")
            nc.gpsimd.partition_broadcast(rsb, rsv)
            return rsb, rsv

        def rope6(pool, pre_ps, cos_t, sin_t, out_ap, nametag, scale_t):
            """rope on 2D psum [P, cols] (head pairs stacked) -> bf16
            out_ap; scale_t (f32 sbuf [P, cols] 1/rms) is folded into the
            evacuation when given. Engines are partition-lane-locked, so
            the half-swap goes through SBUF->SBUF DMA."""
            shp = [int(s) for s in pre_ps.shape]
            kbf = pool.tile(shp, BF16, tag=f"rkb_{nametag}",
                            name=f"rkb_{nametag}")
            if scale_t is None:
                nc.vector.tensor_scalar(kbf, pre_ps, 1.0, None, ALU.mult)
            else:
                nc.vector.tensor_tensor(kbf, pre_ps, scale_t, ALU.mult)
            ksw = pool.tile(shp, BF16, tag=f"rsw_{nametag}",
                            name=f"rsw_{nametag}")
            for g in range(4):
                a = g * HALF
                pa = (g + 1) * HALF if g % 2 == 0 else (g - 1) * HALF
                nc.sync.dma_start(ksw[a:a + HALF], kbf[pa:pa + HALF])
            tmp = pool.tile(shp, BF16, tag=f"rtm_{nametag}",
                            name=f"rtm_{nametag}")
            nc.gpsimd.tensor_tensor(tmp, ksw, sin_t, ALU.mult)
            nc.vector.tensor_tensor(out_ap, kbf, cos_t, ALU.mult)
            nc.vector.tensor_tensor(out_ap, out_ap, tmp, ALU.add)

        attnT = p_x2.tile([P, NHP, TOK], BF16, tag="attnT", name="attnT")
        qTa = p_x2.tile([P, NHP, TOK], BF16, tag="qTa", name="qTa")

        with ExitStack() as kvctx:
            p_kv = kvctx.enter_context(tc.tile_pool(name="kv", bufs=1))
            vextT = p_kv.tile([P, NB, H, HD1], BF16, tag="vextT",
                              name="vextT")
            cT = p_kv.tile([P, RCH, S], BF16, tag="cT", name="cT")

            # ---- phase 1+2: latent cT = (wdkv^T x) * rsb, windowed ----
            with ExitStack() as s12:
                p_xw = s12.enter_context(tc.tile_pool(name="xw", bufs=2))
                p_n1 = s12.enter_context(tc.tile_pool(name="n1", bufs=2))
                p_wd = s12.enter_context(tc.tile_pool(name="wd", bufs=1))
                pp_12 = s12.enter_context(
                    tc.tile_pool(name="p12", bufs=2, space="PSUM"))
                sb_wdkv = p_wd.tile([P, DCH, R], BF16, tag="wdkv",
                                    name="wdkv")
                nc.sync.dma_start(
                    sb_wdkv, wdkvn[:, :].rearrange("(c p) r -> p c r", p=P))
                for w in range(NW):
                    c0 = w * WIN
                    xbf = p_xw.tile([P, DCH, WIN], BF16, tag="xbf",
                                    name="xbf")
                    nc.sync.dma_start(
                        xbf,
                        xTbf[:, c0:c0 + WIN].rearrange("(c p) t -> p c t",
                                                       p=P))
                    rsb1, _ = rms_factors(
                        p_n1, pp_12,
                        [xbf[:, dch, :] for dch in range(DCH)], WIN, "n1")
                    for rch in range(RCH):
                        cps = pp_12.tile([P, WIN], F32, tag="mm", name="mm")
                        for dch in range(DCH):
                            nc.tensor.matmul(
                                cps,
                                sb_wdkv[:, dch, rch * P:(rch + 1) * P],
                                xbf[:, dch, :],
                                start=(dch == 0), stop=(dch == DCH - 1))
                        nc.vector.tensor_tensor(cT[:, rch, c0:c0 + WIN],
                                                cps, rsb1, ALU.mult)

            # ---- phase 4: v -> vextT (ones in column HD of each head) ----
            with ExitStack() as s4:
                p_wv = s4.enter_context(tc.tile_pool(name="wv", bufs=1))
                pp_4 = s4.enter_context(
                    tc.tile_pool(name="p4", bufs=2, space="PSUM"))
                sb_wuv = p_wv.tile([P, RCH, H * HD], BF16, tag="wuv",
                                   name="wuv")
                nc.sync.dma_start(
                    sb_wuv, wuv[:, :].rearrange("(c p) n -> p c n", p=P))
                for tb2 in range(NB // 2):
                    for nh in range(2):
                        vps = pp_4.tile([P, 2, 512], F32, tag="mm", name="mm")
                        for half in range(2):
                            tb = 2 * tb2 + half
                            for rch in range(RCH):
                                nc.tensor.matmul(
                                    vps[:, half, :],
                                    cT[:, rch, tb * P:(tb + 1) * P],
                                    sb_wuv[:, rch, nh * 512:(nh + 1) * 512],
                                    start=(rch == 0), stop=(rch == RCH - 1))
                        dst = vextT[:, 2 * tb2:2 * tb2 + 2,
                                    nh * 8:(nh + 1) * 8, 0:HD]
                        src = vps[:, :, :].rearrange(
                            "p t (h s) -> p t h s", s=HD)
                        if nh == 0:
                            nc.scalar.copy(dst, src)
                        else:
                            nc.vector.tensor_scalar(dst, src, 1.0, None,
                                                    ALU.mult)
                nc.vector.memset(vextT[:, :, :, HD:HD1], 1.0)

            # ---- phase 5: qT = (wq^T x) * rsb + rope (own tokens) ----
            with ExitStack() as s5:
                p_q = s5.enter_context(tc.tile_pool(name="q", bufs=2))
                p_wq = s5.enter_context(tc.tile_pool(name="wqp", bufs=1))
                pp_5 = s5.enter_context(
                    tc.tile_pool(name="p5", bufs=2, space="PSUM"))
                sb_wq = p_wq.tile([P, DCH, H * HD], BF16, tag="wq", name="wq")
                nc.sync.dma_start(
                    sb_wq, wqn[:, :].rearrange("(c p) n -> p c n", p=P))
                rsbq, rsvq = rms_factors(
                    p_q, pp_5,
                    [sb_xqbf[:, dch, :] for dch in range(DCH)], TOK, "nq")
                rsbq2 = p_q.tile([P, 2, TOK], F32, tag="rsbq2", name="rsbq2")
                for g in range(2):
                    nc.gpsimd.partition_broadcast(rsbq2[:, g, :], rsvq)
                for hp2 in range(NHP // 2):
                    qps = pp_5.tile([P, 2, TOK], F32, tag="mm2", name="mm2")
                    for half in range(2):
                        hc = (2 * hp2 + half) * 2 * HD
                        for dch in range(DCH):
                            nc.tensor.matmul(
                                qps[:, half, :],
                                sb_wq[:, dch, hc:hc + P],
                                sb_xqbf[:, dch, :],
                                start=(dch == 0), stop=(dch == DCH - 1))
                    rope6(p_q,
                          qps[:, :, :].rearrange("p a t -> p (a t)"),
                          sb_cos4q[:, :, :].rearrange("p a t -> p (a t)"),
                          sb_sin4qn[:, :, :].rearrange("p a t -> p (a t)"),
                          qTa[:, 2 * hp2:2 * hp2 + 2, :].rearrange(
                              "p a t -> p (a t)"),
                          "q",
                          rsbq2[:, :, :].rearrange("p a t -> p (a t)"))

            # ---- phase 3+6: flash, software-pipelined over head pairs:
            # scores(hp) feed the scalar exp stream while values(hp-1)
            # and kproj+rope(hp+1) run on tensor/vector ----
            with ExitStack() as s6:
                p_kt = s6.enter_context(tc.tile_pool(name="kt", bufs=2))
                p_kr = s6.enter_context(tc.tile_pool(name="kr", bufs=2))
                p_wk = s6.enter_context(tc.tile_pool(name="wk", bufs=1))
                p_fl = s6.enter_context(tc.tile_pool(name="fl", bufs=2))
                p_e2 = s6.enter_context(tc.tile_pool(name="e2p", bufs=2))
                pp_s = s6.enter_context(
                    tc.tile_pool(name="psc", bufs=3, space="PSUM"))
                pp_o = s6.enter_context(
                    tc.tile_pool(name="po", bufs=1, space="PSUM"))
                sb_wuk = p_wk.tile([P, RCH, H * HD], BF16, tag="wuk",
                                   name="wuk")
                nc.sync.dma_start(
                    sb_wuk, wuk[:, :].rearrange("(c p) n -> p c n", p=P))
                sb_cos4k = p_wk.tile([P, S], BF16, tag="cos4k", name="cos4k")
                nc.sync.dma_start(sb_cos4k, cos4k[:, :])
                sb_sin4kn = p_wk.tile([P, S], BF16, tag="sin4kn",
                                      name="sin4kn")
                nc.sync.dma_start(sb_sin4kn, sin4kn[:, :])

                def kproj(hp):
                    hc = hp * 2 * HD
                    kt = p_kt.tile([P, S], BF16, tag="kTa", name="kTa")
                    for w2i in range(NW // 2):
                        c0 = w2i * 2 * WIN
                        kps = pp_s.tile([P, 2, 512], F32, tag="s2",
                                        name="s2")
                        for half in range(2):
                            cw = c0 + half * WIN
                            for rch in range(RCH):
                                nc.tensor.matmul(
                                    kps[:, half, :],
                                    sb_wuk[:, rch, hc:hc + P],
                                    cT[:, rch, cw:cw + WIN],
                                    start=(rch == 0), stop=(rch == RCH - 1))
                        rope6(p_kr,
                              kps[:, :, :].rearrange("p a t -> p (a t)"),
                              sb_cos4k[:, c0:c0 + 2 * WIN],
                              sb_sin4kn[:, c0:c0 + 2 * WIN],
                              kt[:, c0:c0 + 2 * WIN], "k", None)
                    return kt

                def scores_phase(hp, kt):
                    e2 = p_e2.tile([P, 4, E2W], BF16, tag="e2", name="e2")
                    for jp in range(NB // 2):
                        N = pairN[jp]
                        cc = pairOff[jp]
                        for dj in range(2):
                            j = 2 * jp + dj
                            jc = slice(j * P, (j + 1) * P)
                            s2 = pp_s.tile([P, 2, 512], F32, tag="s2",
                                           name="s2")
                            nc.tensor.matmul(
                                s2[:, 0, 0:N], kt[0:HD, jc],
                                qTa[0:HD, hp, 0:N],
                                start=True, stop=True, tile_position=(0, 0))
                            nc.tensor.matmul(
                                s2[:, 1, 0:N], kt[HD:P, jc],
                                qTa[HD:P, hp, 0:N],
                                start=True, stop=True, tile_position=(64, 0))
                            nc.scalar.activation(
                                e2[:, 2 * dj:2 * dj + 2, cc:cc + N],
                                s2[:, :, 0:N], AF.Exp, scale=0.125)
                        nc.vector.tensor_tensor(
                            e2[:, :, cc + N - P:cc + N].rearrange(
                                "p (a b) q -> p a b q", b=2),
                            e2[:, :, cc + N - P:cc + N].rearrange(
                                "p (a b) q -> p a b q", b=2),
                            sb_mask[:, 2 * jp:2 * jp + 2, :, :],
                            ALU.mult)
                    return e2

                def values_phase(hp, e2):
                    O2 = pp_o.tile([P, 2, 512], F32, tag="O2", name="O2")
                    for g in range(2):
                        for jp in range(NB // 2):
                            N = pairN[jp]
                            cc = pairOff[jp]
                            for dj in range(2):
                                j = 2 * jp + dj
                                ve = vextT[:, j, hp * 2 + g, :]
                                nc.tensor.matmul(
                                    O2[0:HD1, g, 0:N], ve,
                                    e2[:, 2 * dj + g, cc:cc + N],
                                    start=(jp == 0 and dj == 0),
                                    stop=(jp == NB // 2 - 1 and dj == 1),
                                    skip_group_check=True)
                    # normalize: attnT = O[0:64] * (1/l); l sits in row 64
                    sums = p_fl.tile([1, 2, TOK], F32, tag="sums",
                                     name="sums")
                    nc.scalar.copy(sums, O2[HD:HD1, :, 0:TOK])
                    sw = p_fl.tile([P, 2 * TOK // P], F32, tag="sw",
                                   name="sw")
                    nc.sync.dma_start(sw, sums)
                    rw = p_fl.tile([P, 2 * TOK // P], F32, tag="rw",
                                   name="rw")
                    nc.vector.reciprocal(rw, sw)
                    linv = p_fl.tile([1, 2, TOK], F32, tag="linv",
                                     name="linv")
                    nc.sync.dma_start(linv, rw)
                    lb = p_fl.tile([P, 2, TOK], F32, tag="lb", name="lb")
                    nc.gpsimd.partition_broadcast(lb[0:HD, :, :],
                                                  linv, channels=HD)
                    nc.vector.tensor_tensor(attnT[0:HD, hp, :],
                                            O2[0:HD, 0, 0:TOK],
                                            lb[0:HD, 0, :], ALU.mult)
                    a2 = p_fl.tile([HD, TOK], BF16, tag="a2", name="a2")
                    nc.vector.tensor_tensor(a2, O2[0:HD, 1, 0:TOK],
                                            lb[0:HD, 1, :], ALU.mult)
                    # head 2 belongs on partitions 64:128 -> move via DMA
                    nc.sync.dma_start(attnT[HD:P, hp, :], a2)

                kt_cur = kproj(0)
                e2_prev = None
                for hp in range(NHP):
                    e2_cur = scores_phase(hp, kt_cur)
                    if hp >= 1:
                        values_phase(hp - 1, e2_prev)
                    if hp + 1 < NHP:
                        kt_cur = kproj(hp + 1)
                    e2_prev = e2_cur
                values_phase(NHP - 1, e2_prev)

        # ---- phase 7: wo + residual -> x2T ----
        x2T = [p_x2.tile([P, TOK], F32, tag=f"x2T{i}", name=f"x2T{i}")
               for i in range(DCH)]
        with ExitStack() as s7:
            p_wo = s7.enter_context(tc.tile_pool(name="wop", bufs=1))
            pp_wo = s7.enter_context(
                tc.tile_pool(name="pwo", bufs=2, space="PSUM"))
            sb_wo = p_wo.tile([P, DCH, D], BF16, tag="wo", name="wo")
            nc.sync.dma_start(
                sb_wo, wo[:, :].rearrange("(c p) n -> p c n", p=P))
            for dch in range(DCH):
                yps = pp_wo.tile([P, TOK], F32, tag="yps", name="yps")
                for hch in range(DCH):
                    nc.tensor.matmul(
                        yps, sb_wo[:, hch, dch * P:(dch + 1) * P],
                        attnT[:, hch, :],
                        start=(hch == 0), stop=(hch == DCH - 1))
                nc.vector.tensor_tensor(x2T[dch], yps, sb_xq[dch], ALU.add)

        # ================= MoE =================
        with ExitStack() as mctx:
            p_moe = mctx.enter_context(tc.tile_pool(name="moe", bufs=1))
            p_sm = mctx.enter_context(tc.tile_pool(name="sm", bufs=2))

            gatesT = p_moe.tile([E, TOK], BF16, tag="gatesT", name="gatesT")
            with ExitStack() as rctx:
                pp_r = rctx.enter_context(
                    tc.tile_pool(name="pr", bufs=2, space="PSUM"))
                # router scores from raw f32 x2 (selection must be exact);
                # the rmsnorm factor folds in after the matmul, which
                # cannot flip the per-token expert ordering
                scp = pp_r.tile([E, TOK], F32, tag="scp", name="scp")
                for dch in range(DCH):
                    nc.tensor.matmul(scp, sb_wrn[:, dch, :], x2T[dch],
                                     start=(dch == 0),
                                     stop=(dch == DCH - 1))
                rsb2, _ = rms_factors(p_moe, pp_r, x2T, TOK, "n2",
                                      sq_split=8)
                x2n8 = p_moe.tile([P, DCH, TOK], FP8, tag="x2n8",
                                  name="x2n8")
                for dch in range(DCH):
                    eng = nc.vector if dch % 2 == 0 else nc.gpsimd
                    eng.tensor_tensor(x2n8[:, dch, :], x2T[dch], rsb2,
                                      ALU.mult)
                sgin = p_moe.tile([E, TOK], F32, tag="sgin", name="sgin")
                nc.vector.tensor_tensor(sgin, scp, rsb2[0:E, :], ALU.mult)
                sg8 = p_moe.tile([E, TOK], F32, tag="sg8", name="sg8")
                nc.scalar.activation(sg8, sgin, AF.Sigmoid)
                for tb in range(SL):
                    tcs = slice(tb * P, (tb + 1) * P)
                    sgt = pp_r.tile([P, E], F32, tag="sgt", name="sgt")
                    nc.tensor.transpose(sgt, sg8[:, tcs],
                                        sb_ident[0:E, 0:E])
                    tt = p_sm.tile([P, E], F32, tag="tt", name="tt")
                    nc.vector.tensor_tensor(tt, sgt, sb_bias, ALU.add)
                    m1 = p_sm.tile([P, 1], F32, tag="m1", name="m1")
                    nc.vector.tensor_reduce(m1, tt, mybir.AxisListType.X,
                                            ALU.max)
                    e1 = p_sm.tile([P, E], F32, tag="e1", name="e1")
                    nc.vector.tensor_scalar(e1, tt, m1, None, ALU.is_ge)
                    t2 = p_sm.tile([P, E], F32, tag="t2", name="t2")
                    nc.vector.scalar_tensor_tensor(t2, e1, -1e9, tt,
                                                   ALU.mult, ALU.add)
                    m2 = p_sm.tile([P, 1], F32, tag="m2", name="m2")
                    nc.vector.tensor_reduce(m2, t2, mybir.AxisListType.X,
                                            ALU.max)
                    e2g = p_sm.tile([P, E], F32, tag="e2g", name="e2g")
                    nc.vector.tensor_scalar(e2g, t2, m2, None, ALU.is_ge)
                    sel = p_sm.tile([P, E], F32, tag="sel", name="sel")
                    nc.vector.tensor_tensor(sel, e1, e2g, ALU.add)
                    gg = p_sm.tile([P, E], F32, tag="gg", name="gg")
                    nc.vector.tensor_tensor(gg, sgt, sel, ALU.mult)
                    dsum = p_sm.tile([P, 1], F32, tag="dsum", name="dsum")
                    nc.vector.tensor_reduce(dsum, gg, mybir.AxisListType.X,
                                            ALU.add)
                    nc.vector.tensor_scalar(dsum, dsum, 1e-9, None, ALU.add)
                    rcp = p_sm.tile([P, 1], F32, tag="rcp", name="rcp")
                    nc.vector.reciprocal(rcp, dsum)
                    nc.vector.tensor_scalar(gg, gg, rcp, None, ALU.mult)
                    gtp = pp_r.tile([E, P], F32, tag="gtp", name="gtp")
                    nc.tensor.transpose(gtp, gg, sb_ident)
                    nc.scalar.copy(gatesT[:, tcs], gtp)

            # gated expert hidden states, fp8 (VS x h1s*h3*gate)
            h2g = p_moe.tile([P, E, 2, TOK], FP8, tag="h2g", name="h2g")
            with ExitStack() as ectx:
                p_mw = ectx.enter_context(tc.tile_pool(name="mw", bufs=3))
                pp_h = ectx.enter_context(
                    tc.tile_pool(name="phps", bufs=2, space="PSUM"))
                for e in range(E):
                    w13t = p_mw.tile([P, DCH, 2 * MH], FP8, tag="w13t",
                                     name="w13t")
                    nc.sync.dma_start(
                        w13t,
                        w13n[e, :, :].rearrange("(c p) n -> p c n", p=P))
                    ge = p_sm.tile([1, TOK], BF16, tag="ge", name="ge")
                    nc.sync.dma_start(ge, gatesT[e:e + 1, :])
                    gb = p_sm.tile([P, TOK], BF16, tag="gb", name="gb")
                    nc.gpsimd.partition_broadcast(gb, ge)
                    hpre = []
                    for m in range(4):
                        hps = pp_h.tile([P, TOK], F32, tag=f"hps{m}",
                                        name=f"hps{m}")
                        for dp in range(DCH // 2):
                            nc.tensor.matmul(
                                hps,
                                w13t[:, 2 * dp:2 * dp + 2,
                                     m * P:(m + 1) * P],
                                x2n8[:, 2 * dp:2 * dp + 2, :],
                                start=(dp == 0), stop=(dp == DCH // 2 - 1),
                                perf_mode=DR)
                        hpre.append(hps)
                    for m in range(2):
                        sl = p_sm.tile([P, TOK], BF16, tag="sl", name="sl")
                        nc.scalar.activation(sl, hpre[m], AF.Silu,
                                             scale=IWS)
                        tg = p_sm.tile([P, TOK], BF16, tag="tg", name="tg")
                        nc.vector.scalar_tensor_tensor(
                            tg, hpre[m + 2], VS * IWS, sl,
                            ALU.mult, ALU.mult)
                        nc.gpsimd.tensor_tensor(h2g[:, e, m, :], tg, gb,
                                                ALU.mult)

            with ExitStack() as w2ctx:
                p_w2 = w2ctx.enter_context(tc.tile_pool(name="w2p", bufs=1))
                pp_yf = w2ctx.enter_context(
                    tc.tile_pool(name="pyf", bufs=2, space="PSUM"))
                w2all = []
                for e in range(E):
                    t = p_w2.tile([P, 2, D], FP8, tag=f"w2_{e}",
                                  name=f"w2_{e}")
                    nc.sync.dma_start(
                        t, w2s[e, :, :].rearrange("(c p) n -> p c n", p=P))
                    w2all.append(t)
                for dch in range(DCH):
                    yf = pp_yf.tile([P, TOK], F32, tag="yf", name="yf")
                    for e in range(E):
                        nc.tensor.matmul(
                            yf, w2all[e][:, :, dch * P:(dch + 1) * P],
                            h2g[:, e, :, :],
                            start=(e == 0), stop=(e == E - 1),
                            perf_mode=DR)
                    ot = p_sm.tile([P, TOK], F32, tag="ot", name="ot")
                    nc.vector.scalar_tensor_tensor(
                        ot, yf, 1.0 / (WS * VS), x2T[dch],
                        ALU.mult, ALU.add)
                    nc.sync.dma_start(outT[dch * P:(dch + 1) * P, :], ot)

    nc.compile()
    return nc


_NC_CACHE = {}


def _get_nc(S):
    if S not in _NC_CACHE:
        _NC_CACHE[S] = _build(S)
    return _NC_CACHE[S]


def host_prep(x, position_ids, norm1_w, wq, wdkv, wuk, wuv, wo,
              norm2_w, wr, router_bias, w1, w3, w2):
    x = np.asarray(x, np.float32)
    _, S, _ = x.shape
    NB = S // P
    SL = NB // 4

    pos = np.asarray(position_ids, np.int32)
    norm1_w = np.asarray(norm1_w, np.float32)
    norm2_w = np.asarray(norm2_w, np.float32)
    wq_n = (np.asarray(wq, np.float32) * norm1_w[:, None]).astype(BF)
    wdkv_n = (np.asarray(wdkv, np.float32) * norm1_w[:, None]).astype(BF)
    wuk_b = np.asarray(wuk, np.float32).astype(BF)
    wuv_b = np.asarray(wuv, np.float32).astype(BF)
    wo_b = np.asarray(wo, np.float32).astype(BF)
    wr_n = np.ascontiguousarray(np.asarray(wr, np.float32) * norm2_w[:, None])
    w13 = np.concatenate([np.asarray(w1, np.float32),
                          np.asarray(w3, np.float32)], axis=2)
    w13_n = np.ascontiguousarray(
        w13 * norm2_w[None, :, None] * WS).astype(E4)
    w2_b = np.ascontiguousarray(np.asarray(w2, np.float32) * WS).astype(E4)
    bias_b = np.ascontiguousarray(np.broadcast_to(
        np.asarray(router_bias, np.float32)[None, :], (P, E)))
    ident = np.eye(P, dtype=np.float32)

    inv = 1.0 / (THETA ** (np.arange(HALF, dtype=np.float64) / HALF))

    in_maps = []
    slot_blocks_all = []
    for c in range(NCORES):
        b, r = divmod(c, 4)
        slot_blocks = [r + 4 * (SL - 1 - m) for m in range(SL)]
        slot_blocks_all.append(slot_blocks)
        own = np.concatenate(
            [np.arange(g * P, (g + 1) * P) for g in slot_blocks])

        ang = pos[b].astype(np.float64)[:, None] * inv[None, :]
        cosT = np.cos(ang).T.astype(np.float32)
        sinT = np.sin(ang).T.astype(np.float32)
        cos4k_h = np.tile(cosT, (4, 1)).astype(BF)
        sin4kn_h = np.concatenate([-sinT, sinT, -sinT, sinT], 0).astype(BF)
        cos4q_h = np.ascontiguousarray(cos4k_h[:, own])
        sin4qn_h = np.ascontiguousarray(sin4kn_h[:, own])

        xT_h = np.ascontiguousarray(x[b].T)
        xTbf_h = xT_h.astype(BF)
        xTq_h = np.ascontiguousarray(x[b].T[:, own])
        xTqbf_h = xTq_h.astype(BF)

        maskt_h = np.zeros((NB, P, P), np.float32)
        for j in range(NB):
            jm = j % 4
            if jm < r:
                maskt_h[j] = 1.0
            elif jm == r:
                maskt_h[j] = np.triu(np.ones((P, P), np.float32))
        maskt_h = np.repeat(maskt_h[:, None, :, :], 2, axis=1).astype(BF)

        in_maps.append({
            "xTbf": xTbf_h, "xTq": xTq_h, "xTqbf": xTqbf_h,
            "cos4k": cos4k_h, "sin4kn": sin4kn_h,
            "cos4q": cos4q_h, "sin4qn": sin4qn_h,
            "maskt": maskt_h,
            "wqn": wq_n, "wdkvn": wdkv_n, "wuk": wuk_b, "wuv": wuv_b,
            "wo": wo_b, "wrn": wr_n, "bias128": bias_b,
            "w13n": w13_n, "w2s": w2_b, "identf": ident,
        })
    return in_maps, slot_blocks_all


def run(inputs, trace=False):
    x = np.asarray(inputs["x"], np.float32)
    Bx, S, Dx = x.shape
    nc = _get_nc(S)
    in_maps, slot_blocks_all = host_prep(**inputs)
    res = run_bass_kernel_spmd(nc, in_maps, core_ids=list(range(NCORES)),
                               trace=trace)
    out = np.zeros((Bx, S, Dx), np.float32)
    for c in range(NCORES):
        b = c // 4
        oT = np.asarray(res.results[c]["outT"])
        for m, g in enumerate(slot_blocks_all[c]):
            out[b, g * P:(g + 1) * P, :] = oT[:, m * P:(m + 1) * P].T
    return out, res


def kernel(**inputs):
    out, _ = run(inputs)
    return out
